# revision 12
# baseline (speedup 1.0000x reference)
"""DLRM dot-interaction kernel for Trainium2 (8 NeuronCores, batch-sharded).

Per sample b: T = concat(dense[b], embs[b]) -> [27, 128]; Z = T @ T^T;
output = strict upper triangle of Z -> [351] fp32.

Per-core plan (2048 samples, 16 blocks of 128), v6:
  - SWDGE cast-DMA loads input blocks as [128 b, (f,d)] fp16.
  - Per super-block of 4 blocks: [all transposes] then [all Gram matmuls]
    so the PE's HAM clock warms once per ~20us matmul burst instead of
    oscillating per block.
  - PE transposes feature slabs into PSUM; DVE/ACT copy into f-major
    Tt [128 d, f*128+b] fp16 (contiguous copies).
  - Per-sample Gram matmul: lhsT = rhs = strided AP [128 d, 27 f];
    out -> PSUM [27, 27] at partition 32*(b%4), column-slot pitch 32.
  - DVE/ACT copy Z PSUM -> SBUF Zs [(g,m) part, (q,n32)] fp16 (full
    contiguous 512-col chunks); SWDGE bounces Zs to DRAM scratch rows.
  - Per quarter: HWDGE reloads (split sync/scalar) scatter scratch into
    sample-major Zb [(g,q) part, (t,m,n27)]; DVE/ACT pack triu into
    Pk [s, (t,351)] fp32; 4 contiguous-row DMAs write out.
"""

import numpy as np

B, NUM_EMBS, D = 16384, 26, 128
N_CORES = 8
BC = B // N_CORES  # 2048 samples per core
BLK = 128          # samples per block
NF = NUM_EMBS + 1  # 27 features
FP = 32            # feature/column pitch (27 + 5 pad)
NPAIR = NF * (NF - 1) // 2  # 351

_CACHE = {}


def build(bc=BC):
    import concourse.bacc as bacc
    import concourse.mybir as mybir
    from concourse.tile import TileContext
    from concourse.masks import make_identity

    fp16 = mybir.dt.float16
    fp32 = mybir.dt.float32

    nc = bacc.Bacc("TRN2", target_bir_lowering=False, debug=False)
    dense_t = nc.dram_tensor("dense", (bc, D), fp32, kind="ExternalInput")
    embs_t = nc.dram_tensor("embs", (bc, NUM_EMBS, D), fp32, kind="ExternalInput")
    out_t = nc.dram_tensor("out", (bc, NPAIR), fp32, kind="ExternalOutput")

    nblk = bc // BLK
    QBLK = 4             # blocks per quarter / super-block
    nq = BLK // 4        # 32 4-sample groups per block
    QG = 16              # groups per PSUM Z tile (1 bank)

    groups = []
    b = 0
    head = [1, 1, 2]
    while b < nblk:
        sz = min(head.pop(0) if head else 4, nblk - b)
        groups.append((b, sz))
        b += sz
    g_of = {}
    for gs, sz in groups:
        for i in range(sz):
            g_of[gs + i] = (gs, sz)

    off = [0] * NF
    for m in range(1, NF):
        off[m] = off[m - 1] + (NF - m)

    with TileContext(nc) as tc:
        with (
            tc.tile_pool(name="consts", bufs=1) as consts,
            tc.tile_pool(name="xin", bufs=2) as xpool,
            tc.tile_pool(name="tt", bufs=8) as ttpool,
            tc.tile_pool(name="zs", bufs=3) as zspool,
            tc.tile_pool(name="zb", bufs=2) as zbpool,
            tc.tile_pool(name="pk", bufs=2) as pkpool,
            tc.tile_pool(name="tp", bufs=4, space="PSUM") as tppool,
            tc.tile_pool(name="zp", bufs=4, space="PSUM") as zppool,
            tc.tile_pool(name="dscr", bufs=8, space="DRAM") as dpool,
        ):
            ident = consts.tile([128, 128], fp16)
            make_identity(nc, ident)

            dview = dense_t.ap()
            eview = embs_t.ap().rearrange("b f d -> b (f d)")
            oview = out_t.ap()

            X = None
            cp_i = 0

            def do_copy(dst, src):
                nonlocal cp_i
                if cp_i % 2 == 0:
                    nc.vector.tensor_copy(out=dst, in_=src)
                else:
                    nc.scalar.copy(dst, src)
                cp_i += 1

            for qtr in range(nblk // QBLK):
                tts = []
                # ---- phase 1: load + transpose 4 blocks ----
                for blki in range(QBLK):
                    blk = qtr * QBLK + blki
                    gs, gsz = g_of[blk]
                    if blk == gs:
                        X = xpool.tile([BLK, gsz * NF * D], fp16, tag="X")
                        dsrc = dview[gs * BLK : (gs + gsz) * BLK].rearrange(
                            "(t b) d -> b t d", t=gsz
                        )
                        xd = X.rearrange("b (t c) -> b t c", t=gsz)
                        nc.gpsimd.dma_start(out=xd[:, :, 0:D], in_=dsrc)
                        esrc = eview[gs * BLK : (gs + gsz) * BLK].rearrange(
                            "(t b) c -> b t c", t=gsz
                        )
                        nc.gpsimd.dma_start(out=xd[:, :, D:], in_=esrc)
                    xoff = (blk - gs) * NF * D

                    Tt = ttpool.tile([128, NF * D], fp16, tag="Tt")
                    nchunk = (NF + 7) // 8  # 8,8,8,3
                    for ci in range(nchunk):
                        c0 = ci * 8
                        cf = min(8, NF - c0)
                        tp = tppool.tile([128, 8 * BLK], fp16, tag="tp")
                        for j in range(cf):
                            f = c0 + j
                            nc.tensor.transpose(
                                tp[:, j * BLK : (j + 1) * BLK],
                                X[:, xoff + f * D : xoff + (f + 1) * D],
                                ident,
                            )
                        do_copy(Tt[:, c0 * BLK : (c0 + cf) * BLK], tp[:, : cf * BLK])
                    tts.append(Tt)

                # ---- phase 2: per-sample Gram matmuls (dense PE burst) ----
                scr_ts = []
                for blki, Tt in enumerate(tts):
                    Ttr = Tt.rearrange("d (f b) -> d b f", b=BLK)  # [128,128,27]
                    Zs_t = zspool.tile(
                        [128, nq * FP], fp16, tag="Zs", name=f"Zs_{qtr}_{blki}"
                    )
                    for qt in range(0, nq, QG):
                        zp = zppool.tile([128, QG * FP], fp32, tag="zp")
                        for q in range(QG):
                            for g in range(4):
                                bloc = (qt + q) * 4 + g
                                wop = Ttr[:, bloc, :NF]  # [128 d, 27 f]
                                mop = Ttr[:, bloc, :NF]
                                nc.tensor.matmul(
                                    zp[
                                        32 * g : 32 * g + NF,
                                        q * FP : q * FP + NF,
                                    ],
                                    wop,
                                    mop,
                                    start=True,
                                    stop=True,
                                    tile_position=(0, 32 * g),
                                )
                        # full-partition contiguous copy (incl junk cols/rows)
                        do_copy(Zs_t[:, qt * FP : (qt + QG) * FP], zp[:, : QG * FP])
                    # bounce to DRAM scratch: 128 contiguous 2048B rows
                    scr_t = dpool.tile([128, nq * FP], fp16, tag="scr")
                    nc.gpsimd.dma_start(out=scr_t[:, :], in_=Zs_t[:, :])
                    scr_ts.append(scr_t)

                # ---- reload scratch -> sample-major Zb [(g,q), (t,m,n)] ----
                Zb = zbpool.tile([128, QBLK * NF * NF], fp16, tag="Zb")
                zb5 = Zb.rearrange(
                    "(g q) (t m n) -> g q t m n", g=4, t=QBLK, n=NF
                )
                for t, scr_t in enumerate(scr_ts):
                    sct = scr_t.rearrange(
                        "(g m) (q n) -> g q m n", g=4, n=FP
                    )  # m=32 rows (5 junk), n=32 cols (5 junk)
                    for g in range(4):
                        eng = nc.sync if (t * 4 + g) % 2 == 0 else nc.scalar
                        eng.dma_start(out=zb5[g, :, t], in_=sct[g][:, :NF, :NF])

                # ---- pack triu -> Pk fp32 ----
                Pk = pkpool.tile([128, QBLK * NPAIR], fp32, tag="Pk")
                zbp = Zb.rearrange("p (t c) -> p t c", t=QBLK)
                pkp = Pk.rearrange("p (t c) -> p t c", t=QBLK)
                for m in range(NF - 1):
                    ln = NF - 1 - m
                    do_copy(
                        pkp[:, :, off[m] : off[m] + ln],
                        zbp[:, :, m * NF + m + 1 : m * NF + NF],
                    )

                # ---- final out: contiguous 1404B rows ----
                ovq = oview[qtr * QBLK * BLK : (qtr + 1) * QBLK * BLK].rearrange(
                    "(t q g) p -> g q t p", g=4, t=QBLK
                )
                pk4 = Pk.rearrange("(g q) (t c) -> g q t c", g=4, t=QBLK)
                for g in range(4):
                    eng = nc.sync if g % 2 == 0 else nc.scalar
                    eng.dma_start(out=ovq[g], in_=pk4[g])

    nc.compile()
    return nc


def _get(bc=BC):
    if bc not in _CACHE:
        _CACHE[bc] = build(bc)
    return _CACHE[bc]


def kernel(dense: np.ndarray, embs: np.ndarray) -> np.ndarray:
    from concourse import bass_utils

    dense = np.ascontiguousarray(np.asarray(dense, dtype=np.float32))
    embs = np.ascontiguousarray(np.asarray(embs, dtype=np.float32))
    assert dense.shape == (B, D) and embs.shape == (B, NUM_EMBS, D)

    nc = _get()
    dsh = dense.reshape(N_CORES, BC, D)
    esh = embs.reshape(N_CORES, BC, NUM_EMBS, D)
    in_maps = [{"dense": dsh[i], "embs": esh[i]} for i in range(N_CORES)]
    res = bass_utils.run_bass_kernel_spmd(nc, in_maps, core_ids=list(range(N_CORES)))
    return np.concatenate([r["out"] for r in res.results], axis=0)
